# revision 5
# baseline (speedup 1.0000x reference)
"""ASPP pooling head on Trainium2 (Bass/Tile), data-parallel over batch on 8 cores.

Computation per sample:
    pooled = mean(x, spatial)            # [Cin]
    y      = relu((pooled @ W.T) * bn_scale + bn_shift)   # [Cout]
    out    = broadcast(y, spatial)       # [Cout, H, W]

Device kernel per core (2 samples):
    - stream x[b] as [128ch, 2*4096] f32 tiles (4 MiB DMAs), VectorE reduce_sum
      over the free (spatial) axis -> pooled column [128, 2]
    - PE matmuls accumulate pooled.T @ wt into PSUM [128out, 1] per o-block,
      wt = (W * bn_scale / 4096).T folded on host
    - ScalarE activation Relu(psum_broadcast + shift) materializes the
      spatially-broadcast output tile [128, 4096]; DMA to DRAM
"""

import numpy as np

B, CIN, H, W_SP = 16, 2048, 64, 64
COUT = 256
NCORES = 8
BPC = B // NCORES          # samples per core
SP = H * W_SP              # 4096 spatial positions
KCH = CIN // 128           # 16 channel chunks of 128
CPT = 2                    # channel chunks per x tile (4 MiB DMA)
NOB = COUT // 128          # output-channel blocks
BN_EPS = 1e-5

_CACHE = {}


def _build_nc():
    import concourse.bacc as bacc
    import concourse.mybir as mybir
    import concourse.tile as tile

    nc = bacc.Bacc("TRN2", target_bir_lowering=False, debug=False,
                   num_devices=NCORES)
    f32 = mybir.dt.float32
    x = nc.dram_tensor("x", [BPC, CIN, SP], f32, kind="ExternalInput").ap()
    wt = nc.dram_tensor("wt", [CIN, COUT], f32, kind="ExternalInput").ap()
    shift = nc.dram_tensor("shift", [COUT], f32, kind="ExternalInput").ap()
    out = nc.dram_tensor("out", [BPC, COUT, SP], f32, kind="ExternalOutput").ap()

    with tile.TileContext(nc) as tc, \
         tc.tile_pool(name="consts", bufs=1) as consts, \
         tc.tile_pool(name="xin", bufs=4) as xin, \
         tc.tile_pool(name="pooled", bufs=8) as pooledp, \
         tc.tile_pool(name="psum", bufs=2, space="PSUM") as psump, \
         tc.tile_pool(name="bcast", bufs=2) as bcastp:

        # wt laid out [128 (c within chunk), KCH * COUT]; chunk k's o-block ob
        # lives at columns k*COUT + ob*128 ...
        wt_sb = consts.tile([128, KCH * COUT], f32)
        nc.sync.dma_start(wt_sb[:].rearrange("p (k o) -> p k o", k=KCH),
                          wt.rearrange("(k p) o -> p k o", p=128))
        shift_sb = consts.tile([128, NOB], f32)
        nc.sync.dma_start(shift_sb[:], shift.rearrange("(ob p) -> p ob", p=128))

        for b in range(BPC):
            pss = [psump.tile([128, 1], f32, name=f"ps{ob}", tag=f"ps{ob}")
                   for ob in range(NOB)]
            for kt in range(KCH // CPT):
                xt = xin.tile([128, CPT, SP], f32)
                src = x[b, kt * CPT * 128:(kt + 1) * CPT * 128, :] \
                    .rearrange("(c p) s -> p c s", p=128)
                nc.sync.dma_start(xt[:], src)
                pt = pooledp.tile([128, CPT], f32)
                nc.vector.reduce_sum(pt[:], xt[:], axis=mybir.AxisListType.X)
                for c in range(CPT):
                    k = kt * CPT + c
                    for ob in range(NOB):
                        nc.tensor.matmul(
                            pss[ob][:],
                            lhsT=wt_sb[:, k * COUT + ob * 128:
                                       k * COUT + ob * 128 + 128],
                            rhs=pt[:, c:c + 1],
                            start=(k == 0),
                            stop=(k == KCH - 1),
                        )
            for ob in range(NOB):
                bc = bcastp.tile([128, SP], f32)
                nc.scalar.activation(
                    bc[:],
                    pss[ob][:].broadcast_to([128, SP]),
                    mybir.ActivationFunctionType.Relu,
                    bias=shift_sb[:, ob:ob + 1],
                    scale=1.0,
                )
                nc.scalar.dma_start(out[b, ob * 128:(ob + 1) * 128, :], bc[:])

    nc.compile()
    return nc


def _prep_inputs(x, W, gamma, beta, running_mean, running_var):
    scale = np.asarray(gamma, np.float32) / np.sqrt(
        np.asarray(running_var, np.float32) + np.float32(BN_EPS))
    wt = np.ascontiguousarray(
        (np.asarray(W, np.float32) * scale[:, None]).T / np.float32(SP))
    shift = (np.asarray(beta, np.float32)
             - np.asarray(running_mean, np.float32) * scale).astype(np.float32)
    xs = np.ascontiguousarray(np.asarray(x, np.float32)).reshape(
        NCORES, BPC, CIN, SP)
    return [{"x": xs[i], "wt": wt, "shift": shift} for i in range(NCORES)]


def kernel(x, W, gamma, beta, running_mean, running_var):
    from concourse import bass_utils

    if "nc" not in _CACHE:
        _CACHE["nc"] = _build_nc()
    nc = _CACHE["nc"]
    in_maps = _prep_inputs(x, W, gamma, beta, running_mean, running_var)
    res = bass_utils.run_bass_kernel_spmd(nc, in_maps,
                                          core_ids=list(range(NCORES)))
    outs = [res.results[i]["out"] for i in range(NCORES)]
    return np.concatenate(outs, axis=0).reshape(B, COUT, H, W_SP)
